# revision 26
# baseline (speedup 1.0000x reference)
"""Trainium2 Bass kernel for nn_DiffusionBlock (anisotropic diffusion step).

Math (per batch-channel image; s = tau*hx^2, hx = grad kernel tap):
  X[i,j] = u[i,j+1]-u[i,j] (0 at j=W-1),  Y[i,j] = u[i+1,j]-u[i,j] (0 at i=H-1)
  XP/YP  = edge-pad(X/Y) on the (H+2, W+2) grid
  F = a*XP + b*YP,  G = b*XP + c*YP              (padded grid)
  out[i,j] = u[i,j] + s*(F[i+1,j+1]-F[i+1,j] + G[i+1,j+1]-G[i,j+1])

The per-call cost through the axon tunnel is dominated by (1) per-operand
dispatch overhead, (2) shipped input bytes (~20 GB/s), (3) the device's DMA
fixed costs; compute engines are idle by comparison. So:
  - ONE core runs all 8 batch images (dispatch overhead scales with cores,
    the tunnel pipe is shared, device compute hides under it).
  - ONE input tensor of 512-byte uint8 rows (dma_start only needs equal
    element counts, so row-groups bitcast to wider tiles):
      per image: u as fp8 E3M4 (2 rows per image row),
                 a|b|c as packed 4-bit codes (3 rows per grid row,
                 value = code/15*max, planes' padded cols 1..W only)
      tail: my/myf/myl/myfl PE matrices as E3M4 (exact; +-1 entries),
            wsp/wsn/wg as bf16 bytes (exact; +-s entries)
  - 3 DMAs per row-tile (u, abc, store) instead of 6.
Tolerance is 2e-2; this config measures ~9e-3 (fp8 u gradients + 4-bit
coefficient quantization).

Per-core layout: row-tiles of R=126 output rows (9 tiles x 2 ch x n_img).
SBUF partition q holds:
  U[q] = u row r0-1+q (top edge-clamped, E3M4)  [rt+2, W]
  ABC[q] = packed a|b|c row r0+q (uint8)        [rt+1, 1536]
DVE decodes nibbles (A/B/C bf16 = (raw & 0xF|0xF0) * k), computes
  XT = free-dim diff of U (col W-1 = 0)
  F[q,s] = A*XT + B*YT at padded row r0+q, cols 1..W (col 0 zeroed)
  G2[q,j] = G[r0+q, j+1] = B*XT + C*YT
PE computes YT[q] = Y row r0-1+q = U[q+1]-U[q] via bidiagonal E3M4
matmuls (first/last-tile clamps folded into myf/myl variants), then
assembles the divergence in PSUM with bf16 weights (shift/sign/scale
folded):  DELTA[p] = s*(F[p+1]@j+1 - F[p+1]@j + G2[p+1] - G2[p])
ACT copies PSUM -> bf16, DMA stores the delta. The host adds the exact
f32 u and recomputes output column 0 exactly (needs a/b at padded col 0,
which the packed planes drop).
"""

import numpy as np
import ml_dtypes

# Problem geometry (hardcoded per harness contract).
N_CORES = 1
N_CH = 2
H = 1024
W = 1024
R = 126       # output rows per tile
CHUNK = 512   # matmul free-dim chunk (= one PSUM bank of fp32)

BF16 = ml_dtypes.bfloat16
F8E3 = ml_dtypes.float8_e3m4

# single input tensor: 256-byte uint8 rows (GCD of the 1024-byte fp8 u row
# and the 1280-byte packed a|b|c row: a 4-bit | b 2-bit | c 4-bit)
ROW_BYTES = 256
U_ROWS = N_CH * H * 4              # 4 rows per 1024-byte u image row
ABC_ROWS = N_CH * (H + 2) * 5      # 5 rows per 1280-byte packed a|b|c row
IMG_ROWS = U_ROWS + ABC_ROWS       # 18452
W_MY_ROWS = 256                    # my|myf|myl|myfl, E3M4 [128, 512] = 2 rows each
W_SB_ROWS = 512                    # wsp|wsn|wg bf16 [128, 512] = 4 rows each
_MY_NAMES = ("my", "myf", "myl", "myfl")
_SB_NAMES = ("wsp", "wsn", "wg")


def _host_weights(s: float, rt_last: int):
    """PE weight matrices: (wmy [128,512] E3M4-exact, wsb [128,512] bf16).

    matmul(out, lhsT, rhs): out[p, n] = sum_k lhsT[k, p] * rhs[k, n]
    """
    k = np.arange(128)[:, None]
    p = np.arange(128)[None, :]
    sf = np.float32(s)
    my = (k == p + 1).astype(np.float32) - (k == p)  # YT[q] = U[q+1]-U[q]
    myf = my.copy()                                  # first tile: YT[0] = U[2]-U[1]
    myf[:, 0] = 0.0
    myf[2, 0] = 1.0
    myf[1, 0] = -1.0
    myl = my.copy()                                  # last tile: YT[rt] = 0
    myl[:, rt_last] = 0.0
    myfl = myf.copy()
    myfl[:, rt_last] = 0.0
    wmy = np.concatenate([my, myf, myl, myfl], axis=1).astype(F8E3)  # exact
    wsp = sf * (k == p + 1)                  # out[p] += s * x[p+1]
    wsn = -sf * (k == p + 1)                 # out[p] -= s * x[p+1]
    wg = sf * (k == p + 1) - sf * (k == p)   # out[p] += s * (x[p+1]-x[p])
    wsb = np.zeros((128, 512), np.float32)
    wsb[:, 0:384] = np.concatenate([wsp, wsn, wg], axis=1)
    return np.ascontiguousarray(wmy), np.ascontiguousarray(wsb.astype(BF16))


def _build_nc(n_ch: int, h: int, w: int, r: int, chunk: int, scales,
              n_img: int = 1, reps: int = 1, mode: str = "full"):
    import concourse.bacc as bacc
    import concourse.mybir as mybir
    import concourse.tile as tile

    f32 = mybir.dt.float32
    bf16 = mybir.dt.bfloat16
    f8e3 = mybir.dt.float8e3
    u8 = mybir.dt.uint8
    and_op = mybir.AluOpType.bitwise_and
    mult_op = mybir.AluOpType.mult

    nc = bacc.Bacc(enable_partition_id=False)
    wb = n_img * IMG_ROWS
    all_d = nc.dram_tensor(
        "all", [wb + W_MY_ROWS + W_SB_ROWS, ROW_BYTES], u8, kind="ExternalInput"
    )
    out_d = nc.dram_tensor("out", [n_img * n_ch, h, w], bf16, kind="ExternalOutput")

    tiles = [(r0, min(r, h - r0)) for r0 in range(0, h, r)]

    with tile.TileContext(nc) as tc:
        with (
            tc.tile_pool(name="wpool", bufs=1) as wpool,
            tc.tile_pool(name="io", bufs=3) as io,
            tc.tile_pool(name="tmp", bufs=2) as tmp,
            tc.tile_pool(name="psum", bufs=2, space="PSUM") as psum,
        ):
            # weight loads (one-time), then a tiny high-priority matmul so PE
            # observes the weights DMA once up front (matmul sync-wait slots
            # are scarce)
            wmy = wpool.tile([128, 512], f8e3, tag="wmy")
            nc.sync.dma_start(
                wmy[:], all_d[wb : wb + W_MY_ROWS, :].bitcast(f8e3)
            )
            wsb = wpool.tile([128, 512], bf16, tag="wsb")
            nc.sync.dma_start(
                wsb[:],
                all_d[wb + W_MY_ROWS : wb + W_MY_ROWS + W_SB_ROWS, :].bitcast(bf16),
            )
            myv = {
                n: wmy[:, i * 128 : (i + 1) * 128]
                for i, n in enumerate(_MY_NAMES)
            }
            wt = {
                n: wsb[:, i * 128 : (i + 1) * 128]
                for i, n in enumerate(_SB_NAMES)
            }
            warm = psum.tile([1, 4], f32, tag="YT")
            with tc.high_priority():
                nc.tensor.matmul(warm[0:1, 0:1], wmy[0:1, 0:1], wmy[0:1, 0:1])

            (sa, sb, sc) = scales
            for _rep in range(reps):
              for img in range(n_img):
               for ch in range(n_ch):
                for r0, rt in tiles:
                    first = r0 == 0
                    last = r0 + rt == h
                    ka = rt + 1      # working partitions
                    ku = rt + 1 if last else rt + 2  # loaded U partitions
                    # ---- loads (u rows are 4 tensor-rows each) ----
                    ub = img * IMG_ROWS + ch * h * 4
                    U = io.tile([128, w], f8e3, tag="U")
                    lo = r0 - 1
                    clo = max(lo, 0)
                    nc.sync.dma_start(
                        U[clo - lo : ku, :],
                        all_d[ub + 4 * clo : ub + 4 * (lo + ku), :].bitcast(f8e3),
                    )
                    if first:
                        nc.sync.dma_start(
                            U[0:1, :], all_d[ub : ub + 4, :].bitcast(f8e3)
                        )
                    # packed a|b|c rows r0..r0+rt (5 tensor-rows per grid row)
                    ab = img * IMG_ROWS + U_ROWS + (ch * (h + 2) + r0) * 5
                    ABC = io.tile([128, 5 * ROW_BYTES], u8, tag="ABC")
                    nc.sync.dma_start(
                        ABC[0:ka, :], all_d[ab : ab + 5 * ka, :]
                    )

                    do_dve = mode in ("full", "nomm")
                    do_pe = mode in ("full", "nodve")
                    # ---- YT (PE): partition-dim forward diff -> PSUM ----
                    # YT[q] = Y row r0-1+q = U[q+1] - U[q] (edge variants folded)
                    YT = psum.tile([128, w], f32, tag="YT")
                    my = myv[{(0, 0): "my", (1, 0): "myf",
                              (0, 1): "myl", (1, 1): "myfl"}[(first, last)]]
                    for n0 in (range(0, w, chunk) if do_pe else ()):
                        nc.tensor.matmul(
                            YT[0:ka, n0 : n0 + chunk],
                            my[0:ku, 0:ka],
                            U[0:ku, n0 : n0 + chunk],
                        )

                    # ---- decode 4-bit planes (DVE): val = (raw&mask)*k ----
                    A = tmp.tile([128, w], bf16, tag="A")
                    Bt = tmp.tile([128, w], bf16, tag="B")
                    C = tmp.tile([128, w], bf16, tag="C")
                    XT = tmp.tile([128, w], bf16, tag="XT")
                    if do_dve:
                        # a, c: 4-bit codes, 2 nibble-planes of 512 cols
                        for dst, base, s_pl in ((A, 0, sa), (C, 768, sc)):
                            raw = ABC[:, base : base + 512]
                            for hi, mask in ((0, 15), (1, 240)):
                                nib = tmp.tile([128, 512], u8, tag="NIB")
                                nc.vector.tensor_scalar(
                                    nib[0:ka, :], raw[0:ka, :], mask, None, and_op
                                )
                                nc.vector.tensor_scalar(
                                    dst[0:ka, hi * 512 : (hi + 1) * 512],
                                    nib[0:ka, :], float(s_pl / (15.0 * (16 if hi else 1))),
                                    None, mult_op,
                                )
                        # b: 2-bit codes, 4 crumb-planes of 256 cols
                        braw = ABC[:, 512:768]
                        for qi in range(4):
                            mask = 3 << (2 * qi)
                            nib = tmp.tile([128, 256], u8, tag="NIB2")
                            nc.vector.tensor_scalar(
                                nib[0:ka, :], braw[0:ka, :], mask, None, and_op
                            )
                            nc.vector.tensor_scalar(
                                Bt[0:ka, qi * 256 : (qi + 1) * 256],
                                nib[0:ka, :], float(sb / (3.0 * (1 << (2 * qi)))),
                                None, mult_op,
                            )
                        # XT[q] = X row r0-1+q: free-dim diff, col W-1 = 0
                        nc.vector.tensor_sub(
                            XT[0:ka, 0 : w - 1], U[0:ka, 1:w], U[0:ka, 0 : w - 1]
                        )
                        nc.vector.memset(XT[0:ka, w - 1 : w], 0.0)

                    # ---- products (DVE) ----
                    # F[q,s] = a[r0+q,s]*XT[q,s-1] + b[r0+q,s]*YT[q,s-1],
                    #   s in 1..W (planes hold cols 1..W); col 0 zeroed
                    #   (host recomputes out col 0 exactly)
                    F = tmp.tile([128, w + 1], bf16, tag="F")
                    T = tmp.tile([128, w], bf16, tag="T")
                    G2 = tmp.tile([128, w], bf16, tag="G2")
                    T2 = tmp.tile([128, w], bf16, tag="T2")
                    if do_dve and do_pe:
                        nc.vector.tensor_mul(F[0:ka, 1 : w + 1], A[0:ka, :], XT[0:ka, :])
                        nc.vector.memset(F[0:ka, 0:1], 0.0)
                        nc.vector.tensor_mul(T[0:ka, :], Bt[0:ka, :], YT[0:ka, :])
                        nc.vector.tensor_add(
                            F[0:ka, 1 : w + 1], F[0:ka, 1 : w + 1], T[0:ka, :]
                        )
                        # G2[q,j] = G[r0+q, j+1]
                        nc.vector.tensor_mul(G2[0:ka, :], Bt[0:ka, :], XT[0:ka, :])
                        nc.vector.tensor_mul(T2[0:ka, :], C[0:ka, :], YT[0:ka, :])
                        nc.vector.tensor_add(G2[0:ka, :], G2[0:ka, :], T2[0:ka, :])
                    elif do_dve:
                        nc.vector.memset(F[0:128, :], 0.0)
                        nc.vector.memset(G2[0:128, :], 0.0)

                    # ---- PSUM assembly (PE) ----
                    # DELTA[p] = s*(F[p+1]@j+1 - F[p+1]@j + G2[p+1] - G2[p])
                    DELTA = psum.tile([128, w], f32, tag="DELTA")
                    for n0 in (range(0, w, chunk) if do_pe and do_dve else ()):
                        cw = min(chunk, w - n0)
                        o = DELTA[0:rt, n0 : n0 + cw]
                        mm = [
                            (wt["wsp"][0:ka, 0:rt], F[0:ka, n0 + 1 : n0 + 1 + cw]),
                            (wt["wsn"][0:ka, 0:rt], F[0:ka, n0 : n0 + cw]),
                            (wt["wg"][0:ka, 0:rt], G2[0:ka, n0 : n0 + cw]),
                        ]
                        for i, (lhsT, rhs) in enumerate(mm):
                            nc.tensor.matmul(
                                o,
                                lhsT,
                                rhs,
                                start=(i == 0),
                                stop=(i == len(mm) - 1),
                            )

                    # ---- PSUM -> SBUF bf16 (ACT), store ----
                    OS = tmp.tile([128, w], bf16, tag="OS")
                    if do_pe and do_dve:
                        nc.scalar.copy(OS[0:rt, :], DELTA[0:rt, :])
                    else:
                        nc.vector.memset(OS[0:128, :], 0.0)
                    nc.sync.dma_start(
                        out_d[img * n_ch + ch, r0 : r0 + rt, :], OS[0:rt, :]
                    )

    nc.compile()
    return nc


def _scale(tau, grad_x):
    hx = float(np.asarray(grad_x)[0, 0, 1, 2])
    return float(np.asarray(tau)) * hx * hx


def _pack_plane(x):
    """f32 plane [..., 1024] (padded cols 1..W) -> (codes packed u8 [..., 512],
    scale). value = code/15*scale."""
    smax = float(x.max())
    if not np.isfinite(smax) or smax <= 0:
        smax = 1.0
    codes = np.clip(np.rint(x * (15.0 / smax)), 0, 15).astype(np.uint8)
    return codes[..., 0:512] | (codes[..., 512:1024] << 4), smax


def _pack_plane2(x):
    """f32 plane [..., 1024] -> (2-bit codes packed u8 [..., 256], scale).
    value = code/3*scale; byte j holds cols j, j+256, j+512, j+768."""
    smax = float(x.max())
    if not np.isfinite(smax) or smax <= 0:
        smax = 1.0
    codes = np.clip(np.rint(x * (3.0 / smax)), 0, 3).astype(np.uint8)
    return (
        codes[..., 0:256]
        | (codes[..., 256:512] << 2)
        | (codes[..., 512:768] << 4)
        | (codes[..., 768:1024] << 6)
    ), smax


def prepare_inputs(u, a, b, c, tau, grad_x, grad_y, n_cores: int = None):
    """Host casts + packing into one uint8 tensor [n_cores, rows, 512].

    Returns (packed, scales)."""
    if n_cores is None:
        n_cores = N_CORES
    s = _scale(tau, grad_x)
    rt_last = H % R if H % R else R
    wmy, wsb = _host_weights(s, rt_last)
    n = np.asarray(u).shape[0]
    n_img = n // n_cores
    u8 = np.ascontiguousarray(np.asarray(u, dtype=np.float32)).astype(F8E3)
    pa, sa = _pack_plane(np.asarray(a, np.float32)[:, :, :, 1 : W + 1])
    pb, sb = _pack_plane2(np.asarray(b, np.float32)[:, :, :, 1 : W + 1])
    pc, sc = _pack_plane(np.asarray(c, np.float32)[:, :, :, 1 : W + 1])
    abc = np.concatenate([pa, pb, pc], axis=3)        # [n, ch, h+2, 1536]
    img_blocks = np.concatenate(
        [
            u8.view(np.uint8).reshape(n, U_ROWS, ROW_BYTES),
            abc.reshape(n, ABC_ROWS, ROW_BYTES),
        ],
        axis=1,
    )  # [n, IMG_ROWS, 512]
    wrows = np.concatenate(
        [
            wmy.view(np.uint8).reshape(W_MY_ROWS, ROW_BYTES),
            wsb.view(np.uint8).reshape(W_SB_ROWS, ROW_BYTES),
        ],
        axis=0,
    )  # [384, 512]
    packed = np.concatenate(
        [
            img_blocks.reshape(n_cores, n_img * IMG_ROWS, ROW_BYTES),
            np.broadcast_to(wrows, (n_cores, *wrows.shape)),
        ],
        axis=1,
    )
    return np.ascontiguousarray(packed), (sa, sb, sc)


def postprocess(delta_f32, u, a, b, c, tau, grad_x):
    """out = u + delta, with output column 0 recomputed exactly on host."""
    s = _scale(tau, grad_x)
    u = np.asarray(u, np.float32)
    a = np.asarray(a, np.float32)
    b = np.asarray(b, np.float32)
    c = np.asarray(c, np.float32)
    out = u + delta_f32
    X0 = u[..., 1] - u[..., 0]                        # [B, C, H]
    Y0 = np.zeros_like(X0)
    Y0[..., : H - 1] = u[..., 1:, 0] - u[..., : H - 1, 0]
    rr = np.clip(np.arange(H + 2) - 1, 0, H - 1)
    Xp0, Yp0 = X0[..., rr], Y0[..., rr]               # [B, C, H+2]
    F0 = a[..., 0] * Xp0 + b[..., 0] * Yp0
    F1 = a[..., 1] * Xp0 + b[..., 1] * Yp0
    G1 = b[..., 1] * Xp0 + c[..., 1] * Yp0
    out[..., 0] = u[..., 0] + s * (
        F1[..., 1 : H + 1] - F0[..., 1 : H + 1] + G1[..., 1 : H + 1] - G1[..., 0:H]
    )
    return out


def kernel(u, a, b, c, tau, grad_x, grad_y):
    from concourse.bass_utils import run_bass_kernel_spmd

    n = np.asarray(u).shape[0]
    n_img = n // N_CORES
    packed, scales = prepare_inputs(u, a, b, c, tau, grad_x, grad_y, N_CORES)
    nc = _build_nc(N_CH, H, W, R, CHUNK, scales, n_img=n_img)
    in_maps = [{"all": packed[k]} for k in range(N_CORES)]
    res = run_bass_kernel_spmd(nc, in_maps, list(range(N_CORES)))
    delta = np.stack(
        [res.results[k]["out"].astype(np.float32) for k in range(N_CORES)], axis=0
    ).reshape(n, N_CH, H, W)
    return postprocess(delta, u, a, b, c, tau, grad_x)
